# revision 14
# baseline (speedup 1.0000x reference)
"""Cross-attention without softmax on 8 trn2 NeuronCores.

Reference computes out = (X Wq^T) (C Wk^T)^T (C Wv^T) * D^-0.5 per batch.
With no softmax the product reassociates:

    out_b = X_b @ A_b,   A_b = scale * Wq^T Wk (C_b^T C_b) Wv^T

which collapses the O(Sq*Skv*D) attention into two O(S*D^2) matmuls plus
a few 128x128 products. Sharding: batch (4) x query-half (2) -> 8 cores;
each core redundantly computes its batch's G = C^T C (no collectives).

I/O is pre-cast to bf16 on the host (halves HBM traffic, kills on-chip
casts); accumulation stays fp32 in PSUM. Row-tiles are formed from a
permuted grouping (partition p holds DRAM rows p*r+j) so every DMA runs
2KB-contiguous per partition. G's row-sum and X-row/out-row mapping are
invariant to that permutation.
"""

import os
import sys
import types

import numpy as np

_TRN_REPO = "/opt/trn_rl_repo"
if _TRN_REPO not in sys.path and not any("trn_rl_repo" in p for p in sys.path):
    sys.path.insert(0, _TRN_REPO)

import ml_dtypes  # noqa: E402

import concourse.bass as bass  # noqa: E402
import concourse.mybir as mybir  # noqa: E402
import concourse.tile as tile  # noqa: E402
from concourse import bacc  # noqa: E402
from concourse.bass_utils import run_bass_kernel_spmd  # noqa: E402
from concourse.masks import make_identity  # noqa: E402

B, SQ, SKV, D = 4, 4096, 4096, 128
N_CORES = 8
SQ_SHARD = SQ // (N_CORES // B)  # 2048
SCALE = float(D) ** -0.5
F32 = mybir.dt.float32
BF16 = mybir.dt.bfloat16

# "bf16": bf16 I/O + compute (fp32 PSUM accum). "f32": fp32 everywhere.
COMPUTE = os.environ.get("KERNEL_COMPUTE", "bf16")

_CACHE: dict = {}


def _install_axon_ntff_shim():
    try:
        import antenv.axon_hooks  # noqa: F401

        return
    except Exception:
        pass
    try:
        from trn_agent_boot.trn_boot import _ntff_profile_via_ctypes

        import antenv

        hook = _ntff_profile_via_ctypes("/opt/axon/libaxon_pjrt.so")
        mod = types.ModuleType("antenv.axon_hooks")
        mod._hook = hook
        mod.get_axon_ntff_profile_hook = lambda: mod._hook

        def _set(h):
            mod._hook = h

        mod.set_axon_ntff_profile_hook = _set
        antenv.axon_hooks = mod
        sys.modules["antenv.axon_hooks"] = mod
    except Exception:
        pass

    try:
        import concourse.bass_utils as bu

        bu.upload_artifacts = lambda tmpdir: f"file://{tmpdir}"
    except Exception:
        pass


def build_tile():
    """One SPMD graph, same on all 8 cores. Per-core inputs:
    x (2048,128), ctx (4096,128), wq/wk/wv (128,128); output out (2048,128).
    """
    cdt = BF16 if COMPUTE == "bf16" else F32

    nc = bacc.Bacc(None, target_bir_lowering=False, debug=False)
    x_ext = nc.declare_dram_parameter("x", [SQ_SHARD, D], cdt, isOutput=False)
    c_ext = nc.declare_dram_parameter("ctx", [SKV, D], cdt, isOutput=False)
    wq_ext = nc.declare_dram_parameter("wq", [D, D], cdt, isOutput=False)
    wk_ext = nc.declare_dram_parameter("wk", [D, D], cdt, isOutput=False)
    wv_ext = nc.declare_dram_parameter("wv", [D, D], cdt, isOutput=False)
    out_ext = nc.declare_dram_parameter("out", [SQ_SHARD, D], cdt, isOutput=True)

    R = 8  # rows per partition in the permuted grouping
    CTX_ROWS = 128 * R  # 1024 rows per ctx chunk
    n_ctx_chunks = SKV // CTX_ROWS  # 4
    n_x_chunks = SQ_SHARD // CTX_ROWS  # 2
    OG = 4  # out tiles per store group

    with tile.TileContext(nc) as tc:
        with (
            tc.tile_pool(name="const", bufs=1) as cpool,
            tc.tile_pool(name="ctxp", bufs=4) as ctxpool,
            tc.tile_pool(name="xp", bufs=2) as xpool,
            tc.tile_pool(name="outp", bufs=2) as opool,
            tc.tile_pool(name="psA", bufs=2, space="PSUM") as psA,
            tc.tile_pool(name="psX", bufs=2, space="PSUM") as psX,
            tc.tile_pool(name="psO", bufs=2, space="PSUM") as psO,
        ):
            ident = cpool.tile([128, 128], cdt)
            make_identity(nc, ident[:])

            wq = cpool.tile([D, D], cdt)
            wk = cpool.tile([D, D], cdt)
            wv = cpool.tile([D, D], cdt)

            # ---- G = C^T C ----
            # ctx chunk c: partition p holds rows c*1024 + p*8 .. +7
            # (2KB-contiguous per partition); slice [:, j, :] is a valid
            # 128-row tile of the row-sum.
            g_ps = psA.tile([D, D], F32, tag="chain")
            cc_chunks = []
            for c in range(n_ctx_chunks):
                cc = ctxpool.tile([128, R, D], cdt, tag="ctx")
                src = c_ext[c * CTX_ROWS : (c + 1) * CTX_ROWS, :].rearrange(
                    "(p r) d -> p r d", p=128
                )
                nc.sync.dma_start(cc[:], src)
                cc_chunks.append(cc)
            for c in range(n_ctx_chunks):
                cc = cc_chunks[c]
                for j in range(R):
                    nc.tensor.matmul(
                        g_ps[:],
                        cc[:, j, :],
                        cc[:, j, :],
                        start=(c == 0 and j == 0),
                        stop=(c == n_ctx_chunks - 1 and j == R - 1),
                    )
            gs = cpool.tile([D, D], cdt)
            nc.vector.tensor_copy(gs[:], g_ps[:])

            # ---- x loads (issued early; consumed by transposes) ----
            nc.sync.dma_start(wq[:], wq_ext[:])
            nc.sync.dma_start(wk[:], wk_ext[:])
            nc.sync.dma_start(wv[:], wv_ext[:])
            x_chunks = []
            for c in range(n_x_chunks):
                xc = xpool.tile([128, R, D], cdt, tag="x")
                src = x_ext[c * CTX_ROWS : (c + 1) * CTX_ROWS, :].rearrange(
                    "(p r) d -> p r d", p=128
                )
                nc.sync.dma_start(xc[:], src)
                x_chunks.append(xc)

            # ---- chain: U = Wq^T Wk; UT; WvT; P = G WvT; A = scale*U P ----
            u_ps = psA.tile([D, D], F32, tag="chain")
            nc.tensor.matmul(u_ps[:], wq[:], wk[:], start=True, stop=True)
            us = cpool.tile([D, D], cdt)
            nc.vector.tensor_copy(us[:], u_ps[:])

            ut_ps = psA.tile([D, D], cdt, tag="chain")
            nc.tensor.transpose(ut_ps[:], us[:], ident[:])
            ut = cpool.tile([D, D], cdt)
            nc.vector.tensor_copy(ut[:], ut_ps[:])

            wvt_ps = psA.tile([D, D], cdt, tag="chain")
            nc.tensor.transpose(wvt_ps[:], wv[:], ident[:])
            wvt = cpool.tile([D, D], cdt)
            nc.vector.tensor_copy(wvt[:], wvt_ps[:])

            p_ps = psA.tile([D, D], F32, tag="chain")
            nc.tensor.matmul(p_ps[:], gs[:], wvt[:], start=True, stop=True)
            ps = cpool.tile([D, D], cdt)
            nc.vector.tensor_copy(ps[:], p_ps[:])

            a_ps = psA.tile([D, D], F32, tag="chain")
            nc.tensor.matmul(a_ps[:], ut[:], ps[:], start=True, stop=True)
            a_sb = cpool.tile([D, D], cdt)
            nc.vector.tensor_copy(a_sb[:], a_ps[:])

            # ---- out rows: groups of OG tiles ----
            for c in range(n_x_chunks):
                xc = x_chunks[c]
                for g in range(R // OG):
                    xt_ps = psX.tile([D, OG * 128], cdt, tag="xtp")
                    for j in range(OG):
                        nc.tensor.transpose(
                            xt_ps[:, j * 128 : (j + 1) * 128],
                            xc[:, g * OG + j, :],
                            ident[:],
                        )
                    xt_sb = xpool.tile([D, OG * 128], cdt, tag="xt")
                    nc.vector.tensor_copy(xt_sb[:], xt_ps[:])

                    o_ps = psO.tile([128, OG * D], F32, tag="ops")
                    for j in range(OG):
                        nc.tensor.matmul(
                            o_ps[:, j * D : (j + 1) * D],
                            xt_sb[:, j * 128 : (j + 1) * 128],
                            a_sb[:],
                            start=True,
                            stop=True,
                        )
                    o_sb = opool.tile([128, OG, D], cdt, tag="osb")
                    nc.vector.tensor_copy(
                        o_sb[:].rearrange("p n d -> p (n d)"), o_ps[:]
                    )
                    dst = out_ext[
                        c * CTX_ROWS : (c + 1) * CTX_ROWS, :
                    ].rearrange("(p r) d -> p r d", p=128)[
                        :, g * OG : (g + 1) * OG, :
                    ]
                    nc.sync.dma_start(dst, o_sb[:])

    nc.compile()
    return nc


def build_raw():
    """Hand-scheduled raw-bass version: no Tile start/tail barriers.

    DMA issue is spread over four engines (sync: ctx0/1 + out stores,
    vector: ctx2/3, gpsimd: x0/1, scalar: wq/wk/ident/wv) because each
    HWDGE trigger costs ~600ns of sequencer time. The identity matrix is
    a host-provided input. PE order interleaves X-transpose groups into
    the slots where it would stall waiting for the next ctx chunk.

    Cumulative semaphore schedules (idx = value after the op):
      PE  (s_pe):  U1 UT2 WvT3 Gc0 4-11 Gc1 12-19 Tg1 20-23 Gc2 24-31
                   Tg2 32-35 Gc3 36-43 Tg3 44-47 P48 Tg4 49-52 A53
                   outg1 54-57 g2 58-61 g3 62-65 g4 66-69
      DVE (s_dve): us1 ut2 wvt3 xt1_4 xt2_5 gs6 ps7 xt3_8 a9 xt4_10
                   o1_11 o2_12 o3_13 o4_14

    PSUM banks: b0=G | b1=U,P,A | b2=UT,WvT | b3=xt1,xt2 | b4=xt3,xt4 |
    b5=o1,o4 | b6=o2 | b7=o3. Same-bank PE-write vs DVE-read pairs are
    serialized by the s_dve waits marked below (P10).
    """
    from contextlib import ExitStack

    cdt = BF16 if COMPUTE == "bf16" else F32
    assert cdt is BF16, "raw impl assumes bf16 I/O"

    nc = bacc.Bacc(None, target_bir_lowering=False, debug=False)
    x_ext = nc.declare_dram_parameter("x", [SQ_SHARD, D], cdt, isOutput=False)
    c_ext = nc.declare_dram_parameter("ctx", [SKV, D], cdt, isOutput=False)
    wq_ext = nc.declare_dram_parameter("wq", [D, D], cdt, isOutput=False)
    wk_ext = nc.declare_dram_parameter("wk", [D, D], cdt, isOutput=False)
    wv_ext = nc.declare_dram_parameter("wv", [D, D], cdt, isOutput=False)
    id_ext = nc.declare_dram_parameter("ident", [D, D], cdt, isOutput=False)
    out_ext = nc.declare_dram_parameter("out", [SQ_SHARD, D], cdt, isOutput=True)

    R = 8
    CTX_ROWS = 128 * R  # 1024
    NCC = SKV // CTX_ROWS  # 4 ctx chunks
    NXC = SQ_SHARD // CTX_ROWS  # 2 x chunks

    ctx_view = [
        c_ext[c * CTX_ROWS : (c + 1) * CTX_ROWS, :].rearrange(
            "(p r) d -> p r d", p=128
        )
        for c in range(NCC)
    ]
    x_view = [
        x_ext[c * CTX_ROWS : (c + 1) * CTX_ROWS, :].rearrange(
            "(p r) d -> p r d", p=128
        )
        for c in range(NXC)
    ]
    out_view = [
        out_ext[c * CTX_ROWS : (c + 1) * CTX_ROWS, :].rearrange(
            "(p r) d -> p r d", p=128
        )
        for c in range(NXC)
    ]

    es = ExitStack()
    _n = [0]

    def sb(shape, dt, name=None):
        _n[0] += 1
        return es.enter_context(
            nc.sbuf_tensor(name or f"sb{_n[0]}", shape, dt)
        )

    def pst(shape, dt, name=None):
        _n[0] += 1
        return es.enter_context(
            nc.psum_tensor(name or f"ps{_n[0]}", shape, dt)
        )

    def sem(name):
        return es.enter_context(nc.semaphore(name))

    with es:
        ident = sb([128, 128], cdt, "ident_sb")
        wq = sb([D, D], cdt, "wq_sb")
        wk = sb([D, D], cdt, "wk_sb")
        wv = sb([D, D], cdt, "wv_sb")
        cc = [sb([128, R, D], cdt, f"cc{i}") for i in range(NCC)]
        xch = [sb([128, R, D], cdt, f"xch{i}") for i in range(NXC)]
        gs = sb([D, D], cdt, "gs")
        us = sb([D, D], cdt, "us")
        ut = sb([D, D], cdt, "ut")
        wvt = sb([D, D], cdt, "wvt")
        pss = sb([D, D], cdt, "pss")
        a_sb = sb([D, D], cdt, "a_sb")
        xt_sb = [sb([D, 512], cdt, f"xt_sb{i}") for i in range(4)]
        o_sb = [sb([128, 4, D], cdt, f"o_sb{i}") for i in range(4)]

        g_ps = pst([128, 512], F32)  # b0 (use [:, :128])
        upa_ps = pst([128, 512], F32)  # b1: U [:, :128], P [:,128:256], A [:,256:384]
        tch_ps = pst([128, 1024], cdt)  # b2: UT [:, :128], WvT [:,128:256]
        xt12_ps = pst([128, 1024], cdt)  # b3
        xt34_ps = pst([128, 1024], cdt)  # b4
        o14_ps = pst([128, 512], F32)  # b5
        o2_ps = pst([128, 512], F32)  # b6
        o3_ps = pst([128, 512], F32)  # b7

        s_pe = sem("s_pe")
        s_dve = sem("s_dve")
        s_w = sem("s_w")
        s_c = [sem(f"s_c{i}") for i in range(NCC)]
        s_x = [sem(f"s_x{i}") for i in range(NXC)]
        s_st = sem("s_st")

        with nc.Block() as block:

            @block.sync
            def _(sync):
                nc.sync.dma_start(cc[0][:], ctx_view[0]).then_inc(s_c[0], 16)
                nc.sync.dma_start(cc[1][:], ctx_view[1]).then_inc(s_c[1], 16)
                for g in range(4):
                    nc.sync.wait_ge(s_dve, 11 + g)
                    dst = out_view[g // 2][:, (g % 2) * 4 : (g % 2) * 4 + 4, :]
                    nc.sync.dma_start(dst, o_sb[g][:]).then_inc(s_st, 16)
                nc.sync.wait_ge(s_st, 64)

            @block.gpsimd
            def _(gp):
                nc.gpsimd.dma_start(xch[0][:], x_view[0]).then_inc(s_x[0], 16)
                nc.gpsimd.dma_start(xch[1][:], x_view[1]).then_inc(s_x[1], 16)

            @block.scalar
            def _(sc):
                nc.scalar.dma_start(cc[2][:], ctx_view[2]).then_inc(s_c[2], 16)
                nc.scalar.dma_start(cc[3][:], ctx_view[3]).then_inc(s_c[3], 16)
                nc.scalar.dma_start(wq[:], wq_ext[:]).then_inc(s_w, 16)
                nc.scalar.dma_start(wk[:], wk_ext[:]).then_inc(s_w, 16)
                nc.scalar.dma_start(ident[:], id_ext[:]).then_inc(s_w, 16)
                nc.scalar.dma_start(wv[:], wv_ext[:]).then_inc(s_w, 16)

            @block.tensor
            def _(te):
                # 1: U = Wq^T Wk
                nc.tensor.wait_ge(s_w, 64)
                nc.tensor.matmul(
                    upa_ps[:, :128], wq[:], wk[:], start=True, stop=True
                ).then_inc(s_pe, 1)
                # 2: UT (ident loaded; us copied)
                nc.tensor.wait_ge(s_dve, 1)
                nc.tensor.transpose(tch_ps[:, :128], us[:], ident[:]).then_inc(
                    s_pe, 1
                )
                # 3: WvT (b2 shared with UT: wait ut copy, P10)
                nc.tensor.wait_ge(s_dve, 2)
                nc.tensor.transpose(
                    tch_ps[:, 128:256], wv[:], ident[:]
                ).then_inc(s_pe, 1)

                def gchunk(c):
                    nc.tensor.wait_ge(s_c[c], 16)
                    for j in range(R):
                        nc.tensor.matmul(
                            g_ps[:, :128],
                            cc[c][:, j, :],
                            cc[c][:, j, :],
                            start=(c == 0 and j == 0),
                            stop=(c == NCC - 1 and j == R - 1),
                        ).then_inc(s_pe, 1)

                def tgroup(ps_ap, xc, base):
                    for j in range(4):
                        nc.tensor.transpose(
                            ps_ap[:, j * 128 : (j + 1) * 128],
                            xc[:, base + j, :],
                            ident[:],
                        ).then_inc(s_pe, 1)

                gchunk(0)  # 4..11
                gchunk(1)  # 12..19
                nc.tensor.wait_ge(s_x[0], 16)
                tgroup(xt12_ps[:, :512], xch[0], 0)  # Tg1 20..23
                gchunk(2)  # 24..31
                nc.tensor.wait_ge(s_dve, 4)  # xt1 copied (b3, P10)
                tgroup(xt12_ps[:, 512:], xch[0], 4)  # Tg2 32..35
                gchunk(3)  # 36..43
                nc.tensor.wait_ge(s_x[1], 16)
                tgroup(xt34_ps[:, :512], xch[1], 0)  # Tg3 44..47
                # 48: P = G WvT
                nc.tensor.wait_ge(s_dve, 6)  # gs copied
                nc.tensor.matmul(
                    upa_ps[:, 128:256], gs[:], wvt[:], start=True, stop=True
                ).then_inc(s_pe, 1)
                nc.tensor.wait_ge(s_dve, 8)  # xt3 copied (b4, P10)
                tgroup(xt34_ps[:, 512:], xch[1], 4)  # Tg4 49..52
                # 53: A = UT P
                nc.tensor.wait_ge(s_dve, 7)  # ps copied
                nc.tensor.matmul(
                    upa_ps[:, 256:384], ut[:], pss[:], start=True, stop=True
                ).then_inc(s_pe, 1)
                # 54..69: out groups
                o_banks = [o14_ps, o2_ps, o3_ps, o14_ps]
                for g in range(4):
                    if g == 0:
                        nc.tensor.wait_ge(s_dve, 9)  # a_sb copied
                    if g == 3:
                        nc.tensor.wait_ge(s_dve, 11)  # o1 copied (b5)
                    for j in range(4):
                        nc.tensor.matmul(
                            o_banks[g][:, j * D : (j + 1) * D],
                            xt_sb[g][:, j * 128 : (j + 1) * 128],
                            a_sb[:],
                            start=True,
                            stop=True,
                        ).then_inc(s_pe, 1)

            @block.vector
            def _(ve):
                def vcopy(dst, src, pe_thresh):
                    nc.vector.wait_ge(s_pe, pe_thresh)
                    nc.vector.tensor_copy(dst, src).then_inc(s_dve, 1)

                vcopy(us[:], upa_ps[:, :128], 1)  # 1
                vcopy(ut[:], tch_ps[:, :128], 2)  # 2
                vcopy(wvt[:], tch_ps[:, 128:256], 3)  # 3
                vcopy(xt_sb[0][:], xt12_ps[:, :512], 23)  # 4
                vcopy(xt_sb[1][:], xt12_ps[:, 512:], 35)  # 5
                vcopy(gs[:], g_ps[:, :128], 43)  # 6
                vcopy(pss[:], upa_ps[:, 128:256], 48)  # 7
                vcopy(xt_sb[2][:], xt34_ps[:, :512], 47)  # 8
                vcopy(a_sb[:], upa_ps[:, 256:384], 53)  # 9
                vcopy(xt_sb[3][:], xt34_ps[:, 512:], 52)  # 10
                o_banks = [o14_ps, o2_ps, o3_ps, o14_ps]
                for g in range(4):  # 11..14
                    vcopy(
                        o_sb[g][:].rearrange("p n d -> p (n d)"),
                        o_banks[g][:],
                        57 + 4 * g,
                    )

    nc.compile()
    return nc


def build():
    if os.environ.get("KERNEL_IMPL", "raw") == "raw":
        return build_raw()
    return build_tile()


def _get_nc():
    if "nc" not in _CACHE:
        _CACHE["nc"] = build()
    return _CACHE["nc"]


def _run(inputs: dict, trace: bool = False, **kw):
    np_dt = ml_dtypes.bfloat16 if COMPUTE == "bf16" else np.float32
    context = np.ascontiguousarray(inputs["context"]).astype(np_dt)
    X = np.ascontiguousarray(inputs["X"]).astype(np_dt)
    Wq = (np.ascontiguousarray(inputs["Wq"]).astype(np.float32) * SCALE).astype(np_dt)
    Wk = np.ascontiguousarray(inputs["Wk"]).astype(np_dt)
    Wv = np.ascontiguousarray(inputs["Wv"]).astype(np_dt)

    raw = os.environ.get("KERNEL_IMPL", "raw") == "raw"
    ident = np.eye(D, dtype=np_dt)
    in_maps = []
    for c in range(N_CORES):
        b, h = divmod(c, 2)
        m = {
            "x": np.ascontiguousarray(
                X[b, h * SQ_SHARD : (h + 1) * SQ_SHARD, :]
            ),
            "ctx": np.ascontiguousarray(context[b]),
            "wq": Wq,
            "wk": Wk,
            "wv": Wv,
        }
        if raw:
            m["ident"] = ident
        in_maps.append(m)

    nc = _get_nc()
    res = run_bass_kernel_spmd(
        nc, in_maps, core_ids=list(range(N_CORES)), trace=trace, **kw
    )
    out = np.empty((B, SQ, D), dtype=np.float32)
    for c in range(N_CORES):
        b, h = divmod(c, 2)
        out[b, h * SQ_SHARD : (h + 1) * SQ_SHARD, :] = res.results[c][
            "out"
        ].astype(np.float32)
    return out, res


def kernel(**inputs: np.ndarray) -> np.ndarray:
    if os.environ.get("BASS_TRACE"):
        _install_axon_ntff_shim()
    out, _ = _run(inputs, trace=False)
    return out


if __name__ == "__main__":
    rng = np.random.default_rng(0)
    ins = {
        "context": rng.standard_normal((B, SKV, D)).astype(np.float32),
        "X": rng.standard_normal((B, SQ, D)).astype(np.float32),
        "Wq": (rng.standard_normal((D, D)) / np.sqrt(D)).astype(np.float32),
        "Wk": (rng.standard_normal((D, D)) / np.sqrt(D)).astype(np.float32),
        "Wv": (rng.standard_normal((D, D)) / np.sqrt(D)).astype(np.float32),
    }
    got = kernel(**ins)
    q = ins["X"] @ ins["Wq"].T
    k = ins["context"] @ ins["Wk"].T
    v = ins["context"] @ ins["Wv"].T
    w = np.einsum("bse,bte->bst", q, k) * SCALE
    want = np.einsum("bst,bte->bse", w, v)
    rel = np.linalg.norm(got - want) / np.linalg.norm(want)
    print("rel err vs numpy:", rel)


# revision 15
# speedup vs baseline: 1.1519x; 1.1519x over previous
"""Cross-attention without softmax on 8 trn2 NeuronCores.

Reference computes out = (X Wq^T) (C Wk^T)^T (C Wv^T) * D^-0.5 per batch.
With no softmax the product reassociates:

    out_b = X_b @ A_b,   A_b = scale * Wq^T Wk (C_b^T C_b) Wv^T

which collapses the O(Sq*Skv*D) attention into two O(S*D^2) matmuls plus
a few 128x128 products. Sharding: batch (4) x query-half (2) -> 8 cores;
each core redundantly computes its batch's G = C^T C (no collectives).

I/O is pre-cast to bf16 on the host (halves HBM traffic, kills on-chip
casts); accumulation stays fp32 in PSUM. Row-tiles are formed from a
permuted grouping (partition p holds DRAM rows p*r+j) so every DMA runs
2KB-contiguous per partition. G's row-sum and X-row/out-row mapping are
invariant to that permutation.
"""

import os
import sys
import types

import numpy as np

_TRN_REPO = "/opt/trn_rl_repo"
if _TRN_REPO not in sys.path and not any("trn_rl_repo" in p for p in sys.path):
    sys.path.insert(0, _TRN_REPO)

import ml_dtypes  # noqa: E402

import concourse.bass as bass  # noqa: E402
import concourse.mybir as mybir  # noqa: E402
import concourse.tile as tile  # noqa: E402
from concourse import bacc  # noqa: E402
from concourse.bass_utils import run_bass_kernel_spmd  # noqa: E402
from concourse.masks import make_identity  # noqa: E402

B, SQ, SKV, D = 4, 4096, 4096, 128
N_CORES = 8
SQ_SHARD = SQ // (N_CORES // B)  # 2048
SCALE = float(D) ** -0.5
F32 = mybir.dt.float32
BF16 = mybir.dt.bfloat16

# "bf16": bf16 I/O + compute (fp32 PSUM accum). "f32": fp32 everywhere.
COMPUTE = os.environ.get("KERNEL_COMPUTE", "bf16")

_CACHE: dict = {}


def _install_axon_ntff_shim():
    try:
        import antenv.axon_hooks  # noqa: F401

        return
    except Exception:
        pass
    try:
        from trn_agent_boot.trn_boot import _ntff_profile_via_ctypes

        import antenv

        hook = _ntff_profile_via_ctypes("/opt/axon/libaxon_pjrt.so")
        mod = types.ModuleType("antenv.axon_hooks")
        mod._hook = hook
        mod.get_axon_ntff_profile_hook = lambda: mod._hook

        def _set(h):
            mod._hook = h

        mod.set_axon_ntff_profile_hook = _set
        antenv.axon_hooks = mod
        sys.modules["antenv.axon_hooks"] = mod
    except Exception:
        pass

    try:
        import concourse.bass_utils as bu

        bu.upload_artifacts = lambda tmpdir: f"file://{tmpdir}"
    except Exception:
        pass


def build_tile():
    """One SPMD graph, same on all 8 cores. Per-core inputs:
    x (2048,128), ctx (4096,128), wq/wk/wv (128,128); output out (2048,128).
    """
    cdt = BF16 if COMPUTE == "bf16" else F32

    nc = bacc.Bacc(None, target_bir_lowering=False, debug=False)
    x_ext = nc.declare_dram_parameter("x", [SQ_SHARD, D], cdt, isOutput=False)
    c_ext = nc.declare_dram_parameter("ctx", [SKV, D], cdt, isOutput=False)
    wq_ext = nc.declare_dram_parameter("wq", [D, D], cdt, isOutput=False)
    wk_ext = nc.declare_dram_parameter("wk", [D, D], cdt, isOutput=False)
    wv_ext = nc.declare_dram_parameter("wv", [D, D], cdt, isOutput=False)
    out_ext = nc.declare_dram_parameter("out", [SQ_SHARD, D], cdt, isOutput=True)

    R = 8  # rows per partition in the permuted grouping
    CTX_ROWS = 128 * R  # 1024 rows per ctx chunk
    n_ctx_chunks = SKV // CTX_ROWS  # 4
    n_x_chunks = SQ_SHARD // CTX_ROWS  # 2
    OG = 4  # out tiles per store group

    with tile.TileContext(nc) as tc:
        with (
            tc.tile_pool(name="const", bufs=1) as cpool,
            tc.tile_pool(name="ctxp", bufs=4) as ctxpool,
            tc.tile_pool(name="xp", bufs=2) as xpool,
            tc.tile_pool(name="outp", bufs=2) as opool,
            tc.tile_pool(name="psA", bufs=2, space="PSUM") as psA,
            tc.tile_pool(name="psX", bufs=2, space="PSUM") as psX,
            tc.tile_pool(name="psO", bufs=2, space="PSUM") as psO,
        ):
            ident = cpool.tile([128, 128], cdt)
            make_identity(nc, ident[:])

            wq = cpool.tile([D, D], cdt)
            wk = cpool.tile([D, D], cdt)
            wv = cpool.tile([D, D], cdt)

            # ---- G = C^T C ----
            # ctx chunk c: partition p holds rows c*1024 + p*8 .. +7
            # (2KB-contiguous per partition); slice [:, j, :] is a valid
            # 128-row tile of the row-sum.
            g_ps = psA.tile([D, D], F32, tag="chain")
            cc_chunks = []
            for c in range(n_ctx_chunks):
                cc = ctxpool.tile([128, R, D], cdt, tag="ctx")
                src = c_ext[c * CTX_ROWS : (c + 1) * CTX_ROWS, :].rearrange(
                    "(p r) d -> p r d", p=128
                )
                nc.sync.dma_start(cc[:], src)
                cc_chunks.append(cc)
            for c in range(n_ctx_chunks):
                cc = cc_chunks[c]
                for j in range(R):
                    nc.tensor.matmul(
                        g_ps[:],
                        cc[:, j, :],
                        cc[:, j, :],
                        start=(c == 0 and j == 0),
                        stop=(c == n_ctx_chunks - 1 and j == R - 1),
                    )
            gs = cpool.tile([D, D], cdt)
            nc.vector.tensor_copy(gs[:], g_ps[:])

            # ---- x loads (issued early; consumed by transposes) ----
            nc.sync.dma_start(wq[:], wq_ext[:])
            nc.sync.dma_start(wk[:], wk_ext[:])
            nc.sync.dma_start(wv[:], wv_ext[:])
            x_chunks = []
            for c in range(n_x_chunks):
                xc = xpool.tile([128, R, D], cdt, tag="x")
                src = x_ext[c * CTX_ROWS : (c + 1) * CTX_ROWS, :].rearrange(
                    "(p r) d -> p r d", p=128
                )
                nc.sync.dma_start(xc[:], src)
                x_chunks.append(xc)

            # ---- chain: U = Wq^T Wk; UT; WvT; P = G WvT; A = scale*U P ----
            u_ps = psA.tile([D, D], F32, tag="chain")
            nc.tensor.matmul(u_ps[:], wq[:], wk[:], start=True, stop=True)
            us = cpool.tile([D, D], cdt)
            nc.vector.tensor_copy(us[:], u_ps[:])

            ut_ps = psA.tile([D, D], cdt, tag="chain")
            nc.tensor.transpose(ut_ps[:], us[:], ident[:])
            ut = cpool.tile([D, D], cdt)
            nc.vector.tensor_copy(ut[:], ut_ps[:])

            wvt_ps = psA.tile([D, D], cdt, tag="chain")
            nc.tensor.transpose(wvt_ps[:], wv[:], ident[:])
            wvt = cpool.tile([D, D], cdt)
            nc.vector.tensor_copy(wvt[:], wvt_ps[:])

            p_ps = psA.tile([D, D], F32, tag="chain")
            nc.tensor.matmul(p_ps[:], gs[:], wvt[:], start=True, stop=True)
            ps = cpool.tile([D, D], cdt)
            nc.vector.tensor_copy(ps[:], p_ps[:])

            a_ps = psA.tile([D, D], F32, tag="chain")
            nc.tensor.matmul(a_ps[:], ut[:], ps[:], start=True, stop=True)
            a_sb = cpool.tile([D, D], cdt)
            nc.vector.tensor_copy(a_sb[:], a_ps[:])

            # ---- out rows: groups of OG tiles ----
            for c in range(n_x_chunks):
                xc = x_chunks[c]
                for g in range(R // OG):
                    xt_ps = psX.tile([D, OG * 128], cdt, tag="xtp")
                    for j in range(OG):
                        nc.tensor.transpose(
                            xt_ps[:, j * 128 : (j + 1) * 128],
                            xc[:, g * OG + j, :],
                            ident[:],
                        )
                    xt_sb = xpool.tile([D, OG * 128], cdt, tag="xt")
                    nc.vector.tensor_copy(xt_sb[:], xt_ps[:])

                    o_ps = psO.tile([128, OG * D], F32, tag="ops")
                    for j in range(OG):
                        nc.tensor.matmul(
                            o_ps[:, j * D : (j + 1) * D],
                            xt_sb[:, j * 128 : (j + 1) * 128],
                            a_sb[:],
                            start=True,
                            stop=True,
                        )
                    o_sb = opool.tile([128, OG, D], cdt, tag="osb")
                    nc.vector.tensor_copy(
                        o_sb[:].rearrange("p n d -> p (n d)"), o_ps[:]
                    )
                    dst = out_ext[
                        c * CTX_ROWS : (c + 1) * CTX_ROWS, :
                    ].rearrange("(p r) d -> p r d", p=128)[
                        :, g * OG : (g + 1) * OG, :
                    ]
                    nc.sync.dma_start(dst, o_sb[:])

    nc.compile()
    return nc


def build_raw():
    """Hand-scheduled raw-bass version: no Tile start/tail barriers.

    DMA issue is spread over four engines (sync: ctx0/1 + out stores,
    vector: ctx2/3, gpsimd: x0/1, scalar: wq/wk/ident/wv) because each
    HWDGE trigger costs ~600ns of sequencer time. The identity matrix is
    a host-provided input. PE order interleaves X-transpose groups into
    the slots where it would stall waiting for the next ctx chunk.

    Cumulative semaphore schedules (idx = value after the op):
      PE  (s_pe):  U1 UT2 WvT3 Gc0 4-11 Gc1 12-19 Tg1 20-23 Gc2 24-31
                   Tg2 32-35 Gc3 36-43 Tg3 44-47 P48 Tg4 49-52 A53
                   outg1 54-57 g2 58-61 g3 62-65 g4 66-69
      DVE (s_dve): us1 ut2 wvt3 xt1_4 xt2_5 gs6 ps7 xt3_8 a9 xt4_10
                   o1_11 o2_12 o3_13 o4_14

    PSUM banks: b0=G | b1=U,P,A | b2=UT,WvT | b3=xt1,xt2 | b4=xt3,xt4 |
    b5=o1,o4 | b6=o2 | b7=o3. Same-bank PE-write vs DVE-read pairs are
    serialized by the s_dve waits marked below (P10).
    """
    from contextlib import ExitStack

    cdt = BF16 if COMPUTE == "bf16" else F32
    assert cdt is BF16, "raw impl assumes bf16 I/O"

    nc = bacc.Bacc(None, target_bir_lowering=False, debug=False)
    x_ext = nc.declare_dram_parameter("x", [SQ_SHARD, D], cdt, isOutput=False)
    c_ext = nc.declare_dram_parameter("ctx", [SKV, D], cdt, isOutput=False)
    wq_ext = nc.declare_dram_parameter("wq", [D, D], cdt, isOutput=False)
    wk_ext = nc.declare_dram_parameter("wk", [D, D], cdt, isOutput=False)
    wv_ext = nc.declare_dram_parameter("wv", [D, D], cdt, isOutput=False)
    id_ext = nc.declare_dram_parameter("ident", [D, D], cdt, isOutput=False)
    out_ext = nc.declare_dram_parameter("out", [SQ_SHARD, D], cdt, isOutput=True)

    R = 8
    CTX_ROWS = 128 * R  # 1024
    NCC = SKV // CTX_ROWS  # 4 ctx chunks
    NXC = SQ_SHARD // CTX_ROWS  # 2 x chunks

    ctx_view = [
        c_ext[c * CTX_ROWS : (c + 1) * CTX_ROWS, :].rearrange(
            "(p r) d -> p r d", p=128
        )
        for c in range(NCC)
    ]
    x_view = [
        x_ext[c * CTX_ROWS : (c + 1) * CTX_ROWS, :].rearrange(
            "(p r) d -> p r d", p=128
        )
        for c in range(NXC)
    ]
    out_view = [
        out_ext[c * CTX_ROWS : (c + 1) * CTX_ROWS, :].rearrange(
            "(p r) d -> p r d", p=128
        )
        for c in range(NXC)
    ]

    es = ExitStack()
    _n = [0]

    def sb(shape, dt, name=None):
        _n[0] += 1
        return es.enter_context(
            nc.sbuf_tensor(name or f"sb{_n[0]}", shape, dt)
        )

    def pst(shape, dt, name=None):
        _n[0] += 1
        return es.enter_context(
            nc.psum_tensor(name or f"ps{_n[0]}", shape, dt)
        )

    def sem(name):
        return es.enter_context(nc.semaphore(name))

    with es:
        ident = sb([128, 128], cdt, "ident_sb")
        wq = sb([D, D], cdt, "wq_sb")
        wk = sb([D, D], cdt, "wk_sb")
        wv = sb([D, D], cdt, "wv_sb")
        cc = [sb([128, R, D], cdt, f"cc{i}") for i in range(NCC)]
        xch = [sb([128, R, D], cdt, f"xch{i}") for i in range(NXC)]
        gs = sb([D, D], cdt, "gs")
        us = sb([D, D], cdt, "us")
        ut = sb([D, D], cdt, "ut")
        wvt = sb([D, D], cdt, "wvt")
        pss = sb([D, D], cdt, "pss")
        a_sb = sb([D, D], cdt, "a_sb")
        xt_sb = [sb([D, 512], cdt, f"xt_sb{i}") for i in range(4)]
        o_sb = [sb([128, 4, D], cdt, f"o_sb{i}") for i in range(4)]

        g_ps = pst([128, 512], F32)  # b0 (use [:, :128])
        upa_ps = pst([128, 512], F32)  # b1: U [:, :128], P [:,128:256], A [:,256:384]
        tch_ps = pst([128, 1024], cdt)  # b2: UT [:, :128], WvT [:,128:256]
        xt12_ps = pst([128, 1024], cdt)  # b3
        xt34_ps = pst([128, 1024], cdt)  # b4
        o14_ps = pst([128, 512], F32)  # b5
        o2_ps = pst([128, 512], F32)  # b6
        o3_ps = pst([128, 512], F32)  # b7

        s_pe = sem("s_pe")
        s_dve = sem("s_dve")
        s_w = sem("s_w")
        s_c = [sem(f"s_c{i}") for i in range(NCC)]
        s_x = [sem(f"s_x{i}") for i in range(NXC)]
        s_st = sem("s_st")

        with nc.Block() as block:

            @block.sync
            def _(sync):
                nc.sync.dma_start(cc[0][:], ctx_view[0]).then_inc(s_c[0], 16)
                nc.sync.dma_start(cc[1][:], ctx_view[1]).then_inc(s_c[1], 16)
                for g in range(4):
                    nc.sync.wait_ge(s_dve, 11 + g)
                    dst = out_view[g // 2][:, (g % 2) * 4 : (g % 2) * 4 + 4, :]
                    nc.sync.dma_start(dst, o_sb[g][:]).then_inc(s_st, 16)
                nc.sync.wait_ge(s_st, 64)

            @block.scalar
            def _(sc):
                nc.scalar.dma_start(cc[2][:], ctx_view[2]).then_inc(s_c[2], 16)
                nc.scalar.dma_start(cc[3][:], ctx_view[3]).then_inc(s_c[3], 16)

            @block.gpsimd
            def _(gp):
                nc.gpsimd.dma_start(wq[:], wq_ext[:]).then_inc(s_w, 16)
                nc.gpsimd.dma_start(wk[:], wk_ext[:]).then_inc(s_w, 16)
                nc.gpsimd.dma_start(ident[:], id_ext[:]).then_inc(s_w, 16)
                nc.gpsimd.dma_start(wv[:], wv_ext[:]).then_inc(s_w, 16)
                nc.gpsimd.dma_start(xch[0][:], x_view[0]).then_inc(s_x[0], 16)
                nc.gpsimd.dma_start(xch[1][:], x_view[1]).then_inc(s_x[1], 16)

            @block.tensor
            def _(te):
                def gchunk(c):
                    nc.tensor.wait_ge(s_c[c], 16)
                    for j in range(R):
                        nc.tensor.matmul(
                            g_ps[:, :128],
                            cc[c][:, j, :],
                            cc[c][:, j, :],
                            start=(c == 0 and j == 0),
                            stop=(c == NCC - 1 and j == R - 1),
                        ).then_inc(s_pe, 1)

                def tgroup(ps_ap, xc, base):
                    for j in range(4):
                        nc.tensor.transpose(
                            ps_ap[:, j * 128 : (j + 1) * 128],
                            xc[:, base + j, :],
                            ident[:],
                        ).then_inc(s_pe, 1)

                gchunk(0)  # 1..8
                gchunk(1)  # 9..16
                gchunk(2)  # 17..24
                nc.tensor.wait_ge(s_x[0], 16)
                nc.tensor.wait_ge(s_w, 64)  # ident
                tgroup(xt12_ps[:, :512], xch[0], 0)  # Tg1 25..28
                gchunk(3)  # 29..36
                nc.tensor.wait_ge(s_dve, 1)  # xt1 copied (b3, P10)
                tgroup(xt12_ps[:, 512:], xch[0], 4)  # Tg2 37..40
                # 41: U = Wq^T Wk
                nc.tensor.matmul(
                    upa_ps[:, :128], wq[:], wk[:], start=True, stop=True
                ).then_inc(s_pe, 1)
                nc.tensor.wait_ge(s_x[1], 16)
                tgroup(xt34_ps[:, :512], xch[1], 0)  # Tg3 42..45
                # 46: UT
                nc.tensor.wait_ge(s_dve, 4)  # us copied
                nc.tensor.transpose(tch_ps[:, :128], us[:], ident[:]).then_inc(
                    s_pe, 1
                )
                # 47: WvT (b2 shared with UT: wait ut copy, P10)
                nc.tensor.wait_ge(s_dve, 6)
                nc.tensor.transpose(
                    tch_ps[:, 128:256], wv[:], ident[:]
                ).then_inc(s_pe, 1)
                # 48: P = G WvT (s_dve>=7 covers gs(2) + us-read of b1, P10)
                nc.tensor.wait_ge(s_dve, 7)
                nc.tensor.matmul(
                    upa_ps[:, 128:256], gs[:], wvt[:], start=True, stop=True
                ).then_inc(s_pe, 1)
                nc.tensor.wait_ge(s_dve, 5)  # xt3 copied (b4, P10)
                tgroup(xt34_ps[:, 512:], xch[1], 4)  # Tg4 49..52
                # 53: A = UT P
                nc.tensor.wait_ge(s_dve, 8)  # ps copied
                nc.tensor.matmul(
                    upa_ps[:, 256:384], ut[:], pss[:], start=True, stop=True
                ).then_inc(s_pe, 1)
                # 54..69: out groups
                o_banks = [o14_ps, o2_ps, o3_ps, o14_ps]
                for g in range(4):
                    if g == 0:
                        nc.tensor.wait_ge(s_dve, 9)  # a_sb copied
                    if g == 3:
                        nc.tensor.wait_ge(s_dve, 11)  # o1 copied (b5)
                    for j in range(4):
                        nc.tensor.matmul(
                            o_banks[g][:, j * D : (j + 1) * D],
                            xt_sb[g][:, j * 128 : (j + 1) * 128],
                            a_sb[:],
                            start=True,
                            stop=True,
                        ).then_inc(s_pe, 1)

            @block.vector
            def _(ve):
                def vcopy(dst, src, pe_thresh):
                    nc.vector.wait_ge(s_pe, pe_thresh)
                    nc.vector.tensor_copy(dst, src).then_inc(s_dve, 1)

                vcopy(xt_sb[0][:], xt12_ps[:, :512], 28)  # 1
                vcopy(gs[:], g_ps[:, :128], 36)  # 2
                vcopy(xt_sb[1][:], xt12_ps[:, 512:], 40)  # 3
                vcopy(us[:], upa_ps[:, :128], 41)  # 4
                vcopy(xt_sb[2][:], xt34_ps[:, :512], 45)  # 5
                vcopy(ut[:], tch_ps[:, :128], 46)  # 6
                vcopy(wvt[:], tch_ps[:, 128:256], 47)  # 7
                vcopy(pss[:], upa_ps[:, 128:256], 48)  # 8
                vcopy(a_sb[:], upa_ps[:, 256:384], 53)  # 9
                vcopy(xt_sb[3][:], xt34_ps[:, 512:], 52)  # 10
                o_banks = [o14_ps, o2_ps, o3_ps, o14_ps]
                for g in range(4):  # 11..14
                    vcopy(
                        o_sb[g][:].rearrange("p n d -> p (n d)"),
                        o_banks[g][:],
                        57 + 4 * g,
                    )

    nc.compile()
    return nc


def build():
    if os.environ.get("KERNEL_IMPL", "raw") == "raw":
        return build_raw()
    return build_tile()


def _get_nc():
    if "nc" not in _CACHE:
        _CACHE["nc"] = build()
    return _CACHE["nc"]


def _run(inputs: dict, trace: bool = False, **kw):
    np_dt = ml_dtypes.bfloat16 if COMPUTE == "bf16" else np.float32
    context = np.ascontiguousarray(inputs["context"]).astype(np_dt)
    X = np.ascontiguousarray(inputs["X"]).astype(np_dt)
    Wq = (np.ascontiguousarray(inputs["Wq"]).astype(np.float32) * SCALE).astype(np_dt)
    Wk = np.ascontiguousarray(inputs["Wk"]).astype(np_dt)
    Wv = np.ascontiguousarray(inputs["Wv"]).astype(np_dt)

    raw = os.environ.get("KERNEL_IMPL", "raw") == "raw"
    ident = np.eye(D, dtype=np_dt)
    in_maps = []
    for c in range(N_CORES):
        b, h = divmod(c, 2)
        m = {
            "x": np.ascontiguousarray(
                X[b, h * SQ_SHARD : (h + 1) * SQ_SHARD, :]
            ),
            "ctx": np.ascontiguousarray(context[b]),
            "wq": Wq,
            "wk": Wk,
            "wv": Wv,
        }
        if raw:
            m["ident"] = ident
        in_maps.append(m)

    nc = _get_nc()
    res = run_bass_kernel_spmd(
        nc, in_maps, core_ids=list(range(N_CORES)), trace=trace, **kw
    )
    out = np.empty((B, SQ, D), dtype=np.float32)
    for c in range(N_CORES):
        b, h = divmod(c, 2)
        out[b, h * SQ_SHARD : (h + 1) * SQ_SHARD, :] = res.results[c][
            "out"
        ].astype(np.float32)
    return out, res


def kernel(**inputs: np.ndarray) -> np.ndarray:
    if os.environ.get("BASS_TRACE"):
        _install_axon_ntff_shim()
    out, _ = _run(inputs, trace=False)
    return out


if __name__ == "__main__":
    rng = np.random.default_rng(0)
    ins = {
        "context": rng.standard_normal((B, SKV, D)).astype(np.float32),
        "X": rng.standard_normal((B, SQ, D)).astype(np.float32),
        "Wq": (rng.standard_normal((D, D)) / np.sqrt(D)).astype(np.float32),
        "Wk": (rng.standard_normal((D, D)) / np.sqrt(D)).astype(np.float32),
        "Wv": (rng.standard_normal((D, D)) / np.sqrt(D)).astype(np.float32),
    }
    got = kernel(**ins)
    q = ins["X"] @ ins["Wq"].T
    k = ins["context"] @ ins["Wk"].T
    v = ins["context"] @ ins["Wv"].T
    w = np.einsum("bse,bte->bst", q, k) * SCALE
    want = np.einsum("bst,bte->bse", w, v)
    rel = np.linalg.norm(got - want) / np.linalg.norm(want)
    print("rel err vs numpy:", rel)
